# revision 8
# baseline (speedup 1.0000x reference)
"""BitLinear-1.58 (absmean ternary quantized linear) Trainium2 kernel.

Full-input contract: kernel(x[4,4096,4096] f32, weight[4096,4096] f32)
-> [4,4096,4096] f32, computing x @ Wq.T with
Wq = sign(W) * clip(round(|W|/gamma), 0, 1), gamma = mean(|W|) + 1e-6.

Sharding: data-parallel over tokens. Each of the 8 cores processes 2048
of the 16384 (b, s) rows with the full weight replicated; no collectives.

The scalar quantization threshold thr = gamma/2 is computed on the host
with the exact same jax-on-CPU op the reference uses (jnp.mean of |W|),
so the ternary decision boundary is bit-identical to the reference's.
All O(N^3) compute and the full elementwise quantization run on device.

Math strategy: fp8 DoubleRow matmuls at 2x PE throughput. x is split
exactly into x ~= hi + lo with hi = e4m3(x), lo = e4m3(x - hi)
(residual <= 2^-8 relative, final rel err ~1e-3), and Wq in {-1,0,+1}
is exact in e4m3. Each DoubleRow matmul contracts TWO 128-deep k-slabs
(0.5 cycles per output row); a hi pass and a lo pass over k-slab pairs
accumulate into the same PSUM group, so the full f32-accuracy product
costs half the fp16 PE time.

Per-core pipeline (no DRAM staging, no fp16 intermediates):
  - x loaded f32, transposed k-major on the PE (f32 transpose mode),
    then split from PSUM: ACT casts psum->xhiT (e4m3), DVE computes
    psum - hi -> xloT (e4m3).
  - W loaded f32 per 128-row tile, quantized on DVE in two passes
    (b = (w < -thr), then q = (w > thr) - b -> e4m3, decisions in f32,
    bit-identical to the reference), PE-transposed k-major per 128x128
    tile, copied back PSUM->SBUF by ACT into 512-column n-blocks.
  - Matmul per (n-block, m-tile): 64 DoubleRow fp8 matmuls (2 halves x
    (hi+lo) x 16 k-slab-pairs) accumulate [128, 512] f32 in PSUM,
    evicted by ACT, DMA'd out. W prep for block nb+1 is pumped between
    m-tiles of block nb; x ingest overlaps block 0's matmuls.
"""

from contextlib import ExitStack

import numpy as np

import concourse.bass as bass
import concourse.mybir as mybir
import concourse.tile as tile
from concourse import bacc
from concourse.bass_utils import run_bass_kernel_spmd
from concourse.masks import make_identity

FP32 = mybir.dt.float32
FP16 = mybir.dt.float16
FP8 = mybir.dt.float8e4

P = 128
EPS = 1e-6
N_CORES = 8

# Full-problem dims (hardcoded per harness contract)
B, S, D_IN, D_OUT = 4, 4096, 4096, 4096
M_FULL = B * S
M_LOC = M_FULL // N_CORES

Copy = mybir.ActivationFunctionType.Copy
DoubleRow = mybir.MatmulPerfMode.DoubleRow


def _bitlinear_body(ctx, tc, out_ap, x_ap, w_ap, thr_ap, nthr_ap,
                    M_loc, D_in, D_out):
    nc = tc.nc
    KC = min(1024, D_in)        # f32 chunk (free dim) for DMA + quantize
    G = min(8, D_in // P)       # 128x128 transposes per PSUM group
    KB = D_in // P              # k-slabs of 128
    KP = KB // 2                # k-slab pairs per DoubleRow pass
    MT = M_loc // P             # m-tiles

    # n-block plan: two narrow lead blocks so matmuls start early, then
    # 512-wide steady-state blocks
    if D_out > 512:
        widths = [P, 3 * P] + [512] * ((D_out - 512) // 512)
    else:
        widths = [D_out]
    starts = [sum(widths[:i]) for i in range(len(widths))]
    NB = len(widths)

    stats = ctx.enter_context(tc.tile_pool(name="stats", bufs=1, side="left"))
    thr_b = stats.tile([P, 1], FP32)
    nc.sync.dma_start(thr_b[:], thr_ap)
    nthr_b = stats.tile([P, 1], FP32)
    nc.sync.dma_start(nthr_b[:], nthr_ap)
    ident32 = stats.tile([P, P], FP32)
    make_identity(nc, ident32[:])
    ident16 = stats.tile([P, P], FP16)
    make_identity(nc, ident16[:])

    # streaming pools on the left; long-lived k-major tensors on the right
    xld = ctx.enter_context(tc.tile_pool(name="xld", bufs=2, side="left"))
    wld = ctx.enter_context(tc.tile_pool(name="wld", bufs=2, side="left"))
    bq = ctx.enter_context(tc.tile_pool(name="bq", bufs=2, side="left"))
    qrow = ctx.enter_context(tc.tile_pool(name="qrow", bufs=2, side="left"))
    co = ctx.enter_context(tc.tile_pool(name="co", bufs=3, side="left"))
    xT = ctx.enter_context(tc.tile_pool(name="xT", bufs=1, side="right"))
    wqt = ctx.enter_context(tc.tile_pool(name="wqt", bufs=2, side="right"))
    ps = ctx.enter_context(tc.tile_pool(name="ps", bufs=2, space="PSUM"))
    tp = ctx.enter_context(tc.tile_pool(name="tp", bufs=2, space="PSUM"))
    tw = ctx.enter_context(tc.tile_pool(name="tw", bufs=2, space="PSUM"))

    xhiT = xT.tile([P, KB, M_loc], FP8, name="xhiT")
    xloT = xT.tile([P, KB, M_loc], FP8, name="xloT")

    wq_bufs = {}
    qrow_bufs = {}

    def wq_quant(nb, rt):
        """DMA + quantize one 128-row tile of W (DVE work only)."""
        if rt == 0:
            wq_bufs[nb] = wqt.tile([P, KB, widths[nb]], FP8, tag="wqt",
                                   name=f"wqt{nb % 2}")
        qrow_t = qrow.tile([P, D_in], FP16, tag="qrow")
        qrow_bufs[(nb, rt)] = qrow_t
        r = starts[nb] // P + rt
        for h in range(D_in // KC):
            wt = wld.tile([P, KC], FP32, tag="wld")
            nc.sync.dma_start(wt[:], w_ap[r * P:(r + 1) * P, h * KC:(h + 1) * KC])
            b = bq.tile([P, KC], FP16, tag="bq")
            nc.vector.tensor_scalar(
                b[:], wt[:], nthr_b[:], None, mybir.AluOpType.is_lt)
            nc.vector.scalar_tensor_tensor(
                qrow_t[:, h * KC:(h + 1) * KC], wt[:], thr_b[:], b[:],
                mybir.AluOpType.is_gt, mybir.AluOpType.subtract)

    def wq_transpose(nb, rt):
        """PE-transpose the quantized row tile into block nb's fp8 buffer."""
        wq_t = wq_bufs[nb]
        qrow_t = qrow_bufs.pop((nb, rt))
        for g in range(KB // G):
            twt = tw.tile([P, G, P], FP16, tag="tw")
            for j in range(G):
                k = g * G + j
                nc.tensor.transpose(
                    twt[:, j, :], qrow_t[:, k * P:(k + 1) * P], ident16[:])
            nc.scalar.activation(
                wq_t[:, g * G:(g + 1) * G, rt * P:(rt + 1) * P], twt[:], Copy)

    def ingest(mt):
        """Load one 128-row x tile, transpose k-major, split hi/lo fp8."""
        for q in range(D_in // KC):
            xt = xld.tile([P, KC], FP32, tag="xld")
            nc.sync.dma_start(xt[:], x_ap[mt * P:(mt + 1) * P, q * KC:(q + 1) * KC])
            tpt = tp.tile([P, G, P], FP32, tag="tp")
            for j in range(G):
                nc.tensor.transpose(
                    tpt[:, j, :], xt[:, j * P:(j + 1) * P], ident32[:])
            kb0 = q * G
            hi = xhiT[:, kb0:kb0 + G, mt * P:(mt + 1) * P]
            nc.scalar.activation(hi, tpt[:], Copy)
            nc.vector.scalar_tensor_tensor(
                xloT[:, kb0:kb0 + G, mt * P:(mt + 1) * P], tpt[:], 1.0, hi,
                mybir.AluOpType.mult, mybir.AluOpType.subtract)

    def matmuls(nb, mt):
        width = widths[nb]
        wq_t = wq_bufs[nb]
        nh = 1 if width <= 256 else 2
        oc = width // nh
        pst = ps.tile([P, width], FP32, tag="ps")
        for h in range(nh):
            o = pst[:, h * oc:(h + 1) * oc]
            for si, src in enumerate((xhiT, xloT)):
                for kp in range(KP):
                    nc.tensor.matmul(
                        o,
                        src[:, 2 * kp:2 * kp + 2, mt * P:(mt + 1) * P],
                        wq_t[:, 2 * kp:2 * kp + 2, h * oc:(h + 1) * oc],
                        start=(si == 0 and kp == 0),
                        stop=(si == 1 and kp == KP - 1),
                        perf_mode=DoubleRow,
                    )
        cot = co.tile([P, width], FP32, tag="co")
        if mt % 2 == 0:
            nc.vector.tensor_copy(out=cot[:], in_=pst[:])
        else:
            nc.scalar.activation(cot[:], pst[:], Copy)
        nc.sync.dma_start(
            out_ap[mt * P:(mt + 1) * P, starts[nb]:starts[nb] + width], cot[:])

    # ---- schedule ----
    # Narrow block 0 starts matmuls early; x ingest interleaves with the
    # block-0 sweep. During each block's sweep, the next block's W
    # row-tiles are pumped with the DVE quantize emitted two m-tiles
    # ahead of the PE transposes so the PE never waits on the chain.
    def pump_steps(nxt):
        # (mt_slot -> action) for quant at 3i+2, transpose at 3i+4
        steps = {}
        for i in range(widths[nxt] // P):
            steps[3 * i + 2] = ("q", i)
            steps[3 * i + 4] = ("t", i)
        return steps

    ingest(0)
    for rt in range(widths[0] // P):
        wq_quant(0, rt)
        wq_transpose(0, rt)
    if MT > 1:
        ingest(1)
    matmuls(0, 0)
    if MT > 1:
        matmuls(0, 1)
    steps = pump_steps(1) if NB > 1 else {}
    for mt in range(2, MT):
        ingest(mt)
        if mt in steps:
            op, i = steps[mt]
            (wq_quant if op == "q" else wq_transpose)(1, i)
        matmuls(0, mt)
    for nb in range(1, NB):
        # finish any pump steps that didn't fit in the previous sweep
        for s in sorted(steps):
            if s >= MT:
                op, i = steps[s]
                (wq_quant if op == "q" else wq_transpose)(nb, i)
        steps = pump_steps(nb + 1) if nb + 1 < NB else {}
        for mt in range(MT):
            if mt in steps:
                op, i = steps[mt]
                (wq_quant if op == "q" else wq_transpose)(nb + 1, i)
            matmuls(nb, mt)


def build_nc(M_loc=M_LOC, D_in=D_IN, D_out=D_OUT):
    nc = bacc.Bacc("TRN2", target_bir_lowering=False, debug=False,
                   num_devices=N_CORES)
    x = nc.dram_tensor("x", [M_loc, D_in], FP32, kind="ExternalInput").ap()
    w = nc.dram_tensor("w", [D_out, D_in], FP32, kind="ExternalInput").ap()
    thr = nc.dram_tensor("thr", [P, 1], FP32, kind="ExternalInput").ap()
    nthr = nc.dram_tensor("nthr", [P, 1], FP32, kind="ExternalInput").ap()
    out = nc.dram_tensor("out", [M_loc, D_out], FP32, kind="ExternalOutput").ap()
    with tile.TileContext(nc) as tc:
        with ExitStack() as ctx:
            _bitlinear_body(ctx, tc, out, x, w, thr, nthr,
                            M_loc, D_in, D_out)
    nc.compile()
    return nc


_NC = None


def _get_nc():
    global _NC
    if _NC is None:
        _NC = build_nc()
    return _NC


def _host_threshold(weight: np.ndarray) -> np.float32:
    """gamma/2 with gamma bit-identical to the reference's jax-on-CPU mean."""
    import jax
    import jax.numpy as jnp

    cpu = jax.devices("cpu")[0]
    with jax.default_device(cpu):
        gamma = jnp.mean(jnp.abs(jnp.asarray(weight, dtype=jnp.float32)))
    gamma = np.float32(gamma) + np.float32(EPS)
    return np.float32(gamma * np.float32(0.5))


def kernel(x: np.ndarray, weight: np.ndarray, **_ignored) -> np.ndarray:
    assert x.shape == (B, S, D_IN) and weight.shape == (D_OUT, D_IN)
    xf = np.ascontiguousarray(x.reshape(M_FULL, D_IN).astype(np.float32, copy=False))
    w = np.ascontiguousarray(weight.astype(np.float32, copy=False))
    thr = _host_threshold(w)
    thr_arr = np.full((P, 1), thr, dtype=np.float32)
    nthr_arr = -thr_arr
    nc = _get_nc()
    in_maps = [
        {"x": np.ascontiguousarray(xf[i * M_LOC:(i + 1) * M_LOC]), "w": w,
         "thr": thr_arr, "nthr": nthr_arr}
        for i in range(N_CORES)
    ]
    res = run_bass_kernel_spmd(nc, in_maps, core_ids=list(range(N_CORES)))
    outs = [res.results[i]["out"] for i in range(N_CORES)]
    full = np.concatenate(outs, axis=0)
    if not np.isfinite(full).all():
        # cold-start transient guard: retry once
        res = run_bass_kernel_spmd(nc, in_maps, core_ids=list(range(N_CORES)))
        outs = [res.results[i]["out"] for i in range(N_CORES)]
        full = np.concatenate(outs, axis=0)
    return full.reshape(B, S, D_OUT).astype(np.float32, copy=False)


if __name__ == "__main__":
    # quick smoke on small shapes via CoreSim
    from concourse.bass_interp import CoreSim

    M_loc, D_in, D_out = 256, 512, 512
    nc = build_nc(M_loc=M_loc, D_in=D_in, D_out=D_out)
    rng = np.random.default_rng(0)
    xs = rng.standard_normal((M_loc, D_in), dtype=np.float32)
    ws = rng.standard_normal((D_out, D_in), dtype=np.float32)
    gamma = np.abs(ws).mean(dtype=np.float32) + np.float32(EPS)
    thr = np.float32(gamma * np.float32(0.5))
    sim = CoreSim(nc, require_finite=True, require_nnan=True)
    sim.tensor("x")[:] = xs
    sim.tensor("w")[:] = ws
    sim.tensor("thr")[:] = np.full((P, 1), thr, np.float32)
    sim.tensor("nthr")[:] = np.full((P, 1), -thr, np.float32)
    sim.simulate(check_with_hw=False)
    got = np.array(sim.tensor("out"))

    wq = np.sign(ws) * np.clip(np.round(np.abs(ws / gamma)), None, 1.0)
    exp = xs @ wq.T.astype(np.float32)
    err = np.abs(got - exp).max() / np.abs(exp).max()
    print("sim rel err:", err)
